# revision 1
# baseline (speedup 1.0000x reference)
"""Trainium2 Bass kernel for nn_MultiHeadSelfTokenAttention.

Reference computation (per (b, s) slice, X = hidden[b, s] in [T=128, H=768]):
    q      = X @ Wq + bq                       [T, 12]     (per-token per-head logit)
    scores = q + mask[:, None] * (-10000)
    alpha  = softmax(scores, axis=T)           [T, 12]
    v      = (X @ Wv + bv).reshape(T, 12, 64)
    res    = einsum('th,thd->hd', alpha, v)    [12, 64] -> [768]
    out    = LN(res @ Wo + bo) * gamma + beta  [768]

Key algebraic restructure: the pooled value P = sum_t alpha * V is computed as
    Y[head, h] = sum_t alpha[t, head] * X[t, h]
    P[head, :] = Y[head, :] @ Wv[:, head*64:(head+1)*64] + bv_head
so V ([T, 768] per slice) is never materialized.

v3 (on top of the v2 full-bf16 datapath):
  - BS=8 sent blocks (4 blocks): denser PE sections, fewer cross-engine
    handoffs per unit work, HAM warm-up.
  - startup: block-0 X loads are the FIRST SWDGE items; small const casts
    are interleaved behind them so the PE starts at ~4us, not ~19us.
  - softmax: batched exp on ACT (no per-sent accumulator reads), one DVE
    tensor_reduce for Z, one broadcast tensor_tensor for the normalize.
  - gamma/beta replication runs under the main loop; G-route first half
    runs under the loop too; LN var+sqrt folded into one activation.

Sharding: data-parallel across batch; core b handles hidden_states[b]
(32 sents).  Weights replicated.  No collectives.
"""

import os
import sys
from contextlib import ExitStack

import numpy as np

for _p in ("/opt/trn_rl_repo", "/root/.axon_site/_ro/trn_rl_repo"):
    if os.path.isdir(_p) and _p not in sys.path:
        sys.path.insert(0, _p)

import concourse.bacc as bacc
import concourse.bass as bass
import concourse.tile as tile
from concourse import mybir
from concourse.bass_utils import run_bass_kernel_spmd

F32 = mybir.dt.float32
BF16 = mybir.dt.bfloat16
AF = mybir.ActivationFunctionType
ALU = mybir.AluOpType

HIDDEN = 768
HEADS = 12
B, S, T = 8, 32, 128
HC = HIDDEN // 128  # 6 chunks of the hidden dim
LN_EPS = 1e-5
MASK_NEG = -10000.0
N_CORES = 8
BS = 8  # sents per block
NBLK = S // BS
HBS = 4  # ett/yt granularity (slices)


def build_kernel():
    nc = bacc.Bacc(trn_type="TRN2", target_bir_lowering=False, debug=False)

    hs = nc.dram_tensor("hs", [S, T, HIDDEN], F32, kind="ExternalInput").ap()
    mask = nc.dram_tensor("mask", [S, T], F32, kind="ExternalInput").ap()
    wq = nc.dram_tensor("wq", [HIDDEN, HEADS], F32, kind="ExternalInput").ap()
    bq = nc.dram_tensor("bq", [HEADS], F32, kind="ExternalInput").ap()
    wv = nc.dram_tensor("wv", [HIDDEN, HIDDEN], F32, kind="ExternalInput").ap()
    bv = nc.dram_tensor("bv", [HIDDEN], F32, kind="ExternalInput").ap()
    wo = nc.dram_tensor("wo", [HIDDEN, HIDDEN], F32, kind="ExternalInput").ap()
    bo = nc.dram_tensor("bo", [HIDDEN], F32, kind="ExternalInput").ap()
    gamma = nc.dram_tensor("gamma", [HIDDEN], F32, kind="ExternalInput").ap()
    beta = nc.dram_tensor("beta", [HIDDEN], F32, kind="ExternalInput").ap()
    ident = nc.dram_tensor("ident", [128, 128], F32, kind="ExternalInput").ap()
    out = nc.dram_tensor("out", [S, HIDDEN], F32, kind="ExternalOutput").ap()

    with tile.TileContext(nc) as tc:
        kernel_body(tc, out, hs, mask, wq, bq, wv, bv, wo, bo, gamma, beta, ident)
    nc.compile()
    return nc


def kernel_body(tc, out, hs, mask, wq, bq, wv, bv, wo, bo, gamma, beta, ident):
    nc = tc.nc
    with ExitStack() as ctx:
        consts = ctx.enter_context(tc.tile_pool(name="consts", bufs=1))
        xp = ctx.enter_context(tc.tile_pool(name="x", bufs=3))
        xtp = ctx.enter_context(tc.tile_pool(name="xt", bufs=2))
        smallp = ctx.enter_context(tc.tile_pool(name="small", bufs=2))
        psctx = ExitStack()
        ps_xt = psctx.enter_context(tc.tile_pool(name="ps_xt", bufs=2, space="PSUM"))
        ps_qt = psctx.enter_context(tc.tile_pool(name="ps_qt", bufs=1, space="PSUM"))
        ps_et = psctx.enter_context(tc.tile_pool(name="ps_et", bufs=1, space="PSUM"))
        ps_yt = psctx.enter_context(tc.tile_pool(name="ps_yt", bufs=1, space="PSUM"))
        ps_g = psctx.enter_context(tc.tile_pool(name="ps_g", bufs=2, space="PSUM"))

        # ---- tiles (no DMA yet; SWDGE order is controlled explicitly) ----
        ident_sb = consts.tile([128, 128], BF16, tag="ident")
        wq_sb = consts.tile([128, HC * HEADS], BF16, tag="wq")
        extras_w = consts.tile([2, HEADS], BF16, tag="exw")
        extras_rhs = consts.tile([2, S * T], BF16, tag="exr")
        bo_row = consts.tile([1, HIDDEN], BF16, tag="bo")
        ones_col = consts.tile([1, S], F32, tag="ones")
        ones_bf = consts.tile([1, S], BF16, tag="onesbf")
        g_row = consts.tile([1, HIDDEN], F32, tag="grow")
        b_row = consts.tile([1, HIDDEN], F32, tag="brow")
        bv_sb = consts.tile([128, HC], F32, tag="bv")
        wv_f32 = consts.tile([128, HC * HIDDEN], F32, tag="wvf")
        wo_f32 = consts.tile([128, HC * HIDDEN], F32, tag="wof")
        wv_sb = consts.tile([128, HC * HIDDEN], BF16, tag="wv")
        wo_sb = consts.tile([128, HC * HIDDEN], BF16, tag="wo")
        gamma_rep = consts.tile([S, HIDDEN], F32, tag="grep")
        beta_rep = consts.tile([S, HIDDEN], F32, tag="brep")
        # P^T staging: pt_sb[64h+j, dc*S+s], head(d)=2dc+h, d=head*64+j
        pt_sb = consts.tile([128, HC * S], BF16, tag="pt")
        # Y^T accumulator: per h-chunk [128, S*HEADS], col = s*12 + head
        yt_sb = [
            consts.tile([128, S * HEADS], BF16, tag=f"yt{c}", name=f"yt{c}")
            for c in range(HC)
        ]

        # ---- f32 consts on the scalar HWDGE ring (parallel with SWDGE) ----
        nc.scalar.dma_start(g_row[:], gamma[None, :])
        nc.scalar.dma_start(b_row[:], beta[None, :])
        nc.scalar.dma_start(bv_sb[:], bv.rearrange("(c p) -> p c", p=128))
        nc.scalar.dma_start(wv_f32[:], wv.rearrange("(c p) n -> p c n", p=128))
        nc.scalar.dma_start(wo_f32[:], wo.rearrange("(c p) n -> p c n", p=128))
        nc.vector.memset(ones_col[:], 1.0)
        nc.vector.memset(ones_bf[:], 1.0)
        nc.vector.memset(extras_w[0:1, :], MASK_NEG)
        nc.vector.memset(extras_rhs[:], 1.0)  # row 0 overwritten by mask DMA

        # bf16 weight-cast chunks interleaved into the main loop (on ACT)
        cast_jobs = []
        for w_dst, w_src in ((wv_sb, wv_f32), (wo_sb, wo_f32)):
            for c in range(HC):
                cast_jobs.append(
                    (w_dst[:, c * HIDDEN : (c + 1) * HIDDEN],
                     w_src[:, c * HIDDEN : (c + 1) * HIDDEN])
                )

        # ---- SWDGE (gpsimd cast-DMA) explicit ordering ----------------
        # x block 0 first (in 2-sent slices so transposes start early),
        # then the small consts needed by block 0's transposes/q.
        x_tiles = {}
        x_tiles[0] = xp.tile([128, BS * HIDDEN], BF16, tag="xblk", name="x_blk")
        for g in range(4):
            nc.gpsimd.dma_start(
                x_tiles[0][:, g * 2 * HIDDEN : (g + 1) * 2 * HIDDEN],
                hs[2 * g : 2 * g + 2].rearrange("s t h -> t s h"),
            )
        nc.gpsimd.dma_start(ident_sb[:], ident[:])
        nc.gpsimd.dma_start(wq_sb[:], wq.rearrange("(c p) n -> p c n", p=128))
        nc.gpsimd.dma_start(
            extras_rhs[0:1, :], mask.rearrange("s t -> (s t)")[None, :]
        )
        nc.gpsimd.dma_start(extras_w[1:2, :], bq[None, :])
        nc.gpsimd.dma_start(bo_row[:], bo[None, :])

        def load_x(blk):
            x_tiles[blk] = xp.tile(
                [128, BS * HIDDEN], BF16, tag="xblk", name="x_blk"
            )
            s0 = blk * BS
            nc.gpsimd.dma_start(
                x_tiles[blk][:], hs[s0 : s0 + BS].rearrange("s t h -> t s h")
            )

        # ---------------- pipeline stages -------------------------------
        def stage_a(blk):
            x_blk = x_tiles[blk]
            # X^T block in SBUF: col = s'*768 + hc*128 + j
            xt_blk = xtp.tile([128, BS * HIDDEN], BF16, tag="xtblk", name="xt_blk")
            for sp in range(BS):
                xt_ps = ps_xt.tile([128, HIDDEN], BF16, tag="xtps", name="xt_ps")
                for c in range(HC):
                    nc.tensor.transpose(
                        xt_ps[:, c * 128 : (c + 1) * 128],
                        x_blk[
                            :, sp * HIDDEN + c * 128 : sp * HIDDEN + (c + 1) * 128
                        ],
                        ident_sb[:],
                    )
                nc.vector.tensor_copy(
                    xt_blk[:, sp * HIDDEN : sp * HIDDEN + 384], xt_ps[:, 0:384]
                )
                nc.scalar.copy(
                    xt_blk[:, sp * HIDDEN + 384 : (sp + 1) * HIDDEN],
                    xt_ps[:, 384:768],
                )
            return x_blk, xt_blk

        def stage_q(blk, xt_blk):
            s0 = blk * BS
            qt_ps = ps_qt.tile([HEADS, BS * T], F32, tag="qt", name="qt_ps")
            xt_r = xt_blk.rearrange("p (s c j) -> p c s j", s=BS, j=128)
            spw = 512 // T
            nh = BS // spw
            for c in range(HC):
                for h in range(nh):
                    nc.tensor.matmul(
                        qt_ps[:, h * 512 : (h + 1) * 512],
                        wq_sb[:, c * HEADS : (c + 1) * HEADS],
                        xt_r[:, c, h * spw : (h + 1) * spw],
                        start=(c == 0),
                        stop=False,
                    )
            for h in range(nh):
                nc.tensor.matmul(
                    qt_ps[:, h * 512 : (h + 1) * 512],
                    extras_w[:],
                    extras_rhs[:, s0 * T + h * 512 : s0 * T + (h + 1) * 512],
                    start=False,
                    stop=True,
                )

            # softmax pieces (no max-subtraction: unmasked logits are O(5);
            # masked logits are ~-1e4 and exp underflows to exactly 0)
            et_sb = smallp.tile([HEADS, BS * T], F32, tag="et", name="et_sb")
            for h in range(nh):
                nc.scalar.activation(
                    et_sb[:, h * 512 : (h + 1) * 512],
                    qt_ps[:, h * 512 : (h + 1) * 512],
                    AF.Exp,
                )
            zsum = smallp.tile([HEADS, BS], F32, tag="zsum", name="zsum")
            nc.vector.tensor_reduce(
                zsum[:],
                et_sb.rearrange("p (s t) -> p s t", t=T),
                axis=mybir.AxisListType.X,
                op=ALU.add,
            )
            zinv = smallp.tile([HEADS, BS], F32, tag="zinv", name="zinv")
            nc.vector.reciprocal(zinv[:], zsum[:])
            return et_sb, zinv

        def stage_b(blk, x_blk, et_sb, zinv):
            s0 = blk * BS
            # normalize: alpha^T = e^T * (1/Z), broadcast along t
            at_sb = smallp.tile([HEADS, BS * T], BF16, tag="at", name="at_sb")
            nc.vector.tensor_tensor(
                at_sb.rearrange("p (s t) -> p s t", t=T),
                et_sb.rearrange("p (s t) -> p s t", t=T),
                zinv[:, :, None].broadcast_to([HEADS, BS, T]),
                op=ALU.mult,
            )
            for half in range(BS // HBS):
                ett_ps = ps_et.tile(
                    [128, HBS * HEADS], BF16, tag="ett", name="ett_ps"
                )
                for hp in range(HBS):
                    sp = half * HBS + hp
                    nc.tensor.transpose(
                        ett_ps[:, hp * HEADS : (hp + 1) * HEADS],
                        at_sb[:, sp * T : (sp + 1) * T],
                        ident_sb[0:HEADS, 0:HEADS],
                    )
                e_sb = smallp.tile([128, HBS * HEADS], BF16, tag="e", name="e_sb")
                nc.vector.tensor_copy(e_sb[:], ett_ps[:])

                yt_ps = ps_yt.tile(
                    [128, HC * HBS * HEADS], F32, tag="ytps", name="yt_ps"
                )
                for hp in range(HBS):
                    sp = half * HBS + hp
                    for c in range(HC):
                        nc.tensor.matmul(
                            yt_ps[
                                :,
                                c * HBS * HEADS
                                + hp * HEADS : c * HBS * HEADS
                                + (hp + 1) * HEADS,
                            ],
                            x_blk[
                                :,
                                sp * HIDDEN + c * 128 : sp * HIDDEN + (c + 1) * 128,
                            ],
                            e_sb[:, hp * HEADS : (hp + 1) * HEADS],
                        )
                for c in range(HC):
                    eng = nc.vector.tensor_copy if c % 2 == 0 else nc.scalar.copy
                    eng(
                        yt_sb[c][
                            :,
                            (s0 + half * HBS)
                            * HEADS : (s0 + half * HBS + HBS)
                            * HEADS,
                        ],
                        yt_ps[:, c * HBS * HEADS : (c + 1) * HBS * HEADS],
                    )

        # G-route for a range of sents: G^T[d, (s,head)] = Wv^T-chunks @ Y^T;
        # per-head diagonal extracted: P^T[d, s] = G^T[d, s*12+head(d)] + bv[d]
        def g_route(sent0, nsent):
            c0 = sent0 * HEADS
            c1 = (sent0 + nsent) * HEADS
            for dc in range(HC):
                g_ps = ps_g.tile([128, nsent * HEADS], F32, tag="g", name="g_ps")
                for c in range(HC):
                    nc.tensor.matmul(
                        g_ps[:],
                        wv_sb[
                            :, c * HIDDEN + dc * 128 : c * HIDDEN + (dc + 1) * 128
                        ],
                        yt_sb[c][:, c0:c1],
                        start=(c == 0),
                        stop=(c == HC - 1),
                    )
                g_r = g_ps.rearrange("p (s n) -> p s n", n=HEADS)
                for half in range(2):
                    head = 2 * dc + half
                    rows = slice(half * 64, half * 64 + 64)
                    dst = pt_sb[rows, dc * S + sent0 : dc * S + sent0 + nsent]
                    if half == 0:
                        nc.vector.tensor_scalar_add(
                            dst, g_r[rows, :, head], bv_sb[rows, dc : dc + 1]
                        )
                    else:
                        nc.scalar.activation(
                            dst,
                            g_r[rows, :, head],
                            AF.Identity,
                            bias=bv_sb[rows, dc : dc + 1],
                        )

        # ---------------- main loop --------------------------------------
        stash = {}
        for blk in range(NBLK):
            if blk + 1 < NBLK:
                load_x(blk + 1)
            a = stage_a(blk)
            if blk == 1:
                # replicate gamma/beta across the 32 sent-partitions while
                # the PE is mid-loop (K=1 matmuls + copies)
                for row, rep in ((g_row, gamma_rep), (b_row, beta_rep)):
                    gb1 = ps_g.tile([S, 512], F32, tag="g", name="gb1")
                    gb2 = ps_g.tile([S, 256], F32, tag="g", name="gb2")
                    nc.tensor.matmul(gb1[:], ones_col[:], row[:, 0:512])
                    nc.tensor.matmul(gb2[:], ones_col[:], row[:, 512:768])
                    nc.vector.tensor_copy(rep[:, 0:512], gb1[:])
                    nc.scalar.copy(rep[:, 512:768], gb2[:])
            if blk - 1 in stash:
                stage_b(blk - 1, *stash.pop(blk - 1))
            if blk == 2:
                g_route(0, 2 * BS)  # first half of G under the loop
            et_sb, zinv = stage_q(blk, a[1])
            stash[blk] = (a[0], et_sb, zinv)
            for _ in range(3):
                if cast_jobs:
                    dst, src = cast_jobs.pop(0)
                    nc.scalar.copy(dst, src)
            if blk == 3:
                # third G quarter fills the PE idle while softmax(3) makes
                # its cross-engine round-trip (no next-block transposes left
                # to cover it)
                g_route(2 * BS, BS)
                # pre-warm the Square/Sqrt activation tables here (ACT has
                # slack) so the LN tail skips its ACT_TABLE_LOAD + drain
                warm = smallp.tile([1, 2], F32, tag="warm", name="warm")
                nc.scalar.activation(warm[0:1, 0:1], ones_col[0:1, 0:1], AF.Square)
                nc.scalar.activation(warm[0:1, 1:2], ones_col[0:1, 0:1], AF.Sqrt)
        stage_b(NBLK - 1, *stash.pop(NBLK - 1))
        while cast_jobs:
            dst, src = cast_jobs.pop(0)
            nc.scalar.copy(dst, src)
        g_route(3 * BS, BS)  # last G quarter

        psctx.close()  # free the main-loop PSUM banks

        # ---------------- output projection + layernorm -------------------
        with (
            tc.tile_pool(name="ps_o", bufs=1, space="PSUM") as ps_o,
            tc.tile_pool(name="fin", bufs=1) as fin,
        ):
            # out = P @ Wo + bo   -> [32, 768]
            o1 = ps_o.tile([S, 512], F32, tag="o1", name="o1")
            o2 = ps_o.tile([S, 256], F32, tag="o2", name="o2")
            for dc in range(HC):
                nc.tensor.matmul(
                    o1[:],
                    pt_sb[:, dc * S : (dc + 1) * S],
                    wo_sb[:, dc * HIDDEN : dc * HIDDEN + 512],
                    start=(dc == 0),
                    stop=False,
                )
                nc.tensor.matmul(
                    o2[:],
                    pt_sb[:, dc * S : (dc + 1) * S],
                    wo_sb[:, dc * HIDDEN + 512 : (dc + 1) * HIDDEN],
                    start=(dc == 0),
                    stop=False,
                )
            nc.tensor.matmul(
                o1[:], ones_bf[:], bo_row[:, 0:512], start=False, stop=True
            )
            nc.tensor.matmul(
                o2[:], ones_bf[:], bo_row[:, 512:768], start=False, stop=True
            )

            res_sb = fin.tile([S, HIDDEN], F32, tag="res", name="res_sb")
            mu_parts = fin.tile([S, 2], F32, tag="mup", name="mu_parts")
            nc.scalar.activation(
                res_sb[:, 0:512], o1[:], AF.Copy, accum_out=mu_parts[:, 0:1]
            )
            nc.scalar.activation(
                res_sb[:, 512:768], o2[:], AF.Copy, accum_out=mu_parts[:, 1:2]
            )
            mu = fin.tile([S, 1], F32, tag="mu", name="mu")
            nc.vector.tensor_reduce(
                mu[:], mu_parts[:], axis=mybir.AxisListType.X, op=ALU.add
            )
            muv = fin.tile([S, 1], F32, tag="muv", name="muv")
            nc.vector.tensor_scalar_mul(muv[:], mu[:], 1.0 / HIDDEN)
            xc = fin.tile([S, HIDDEN], F32, tag="xc", name="xc")
            nc.vector.tensor_scalar_sub(xc[:], res_sb[:], muv[:])
            sq = fin.tile([S, HIDDEN], F32, tag="sq", name="sq")
            varsum = fin.tile([S, 1], F32, tag="vs", name="varsum")
            nc.scalar.activation(sq[:], xc[:], AF.Square, accum_out=varsum[:])
            vareps = fin.tile([S, 1], F32, tag="ve", name="vareps")
            nc.vector.tensor_scalar(
                vareps[:], varsum[:], 1.0 / HIDDEN, LN_EPS, op0=ALU.mult, op1=ALU.add
            )
            sd = fin.tile([S, 1], F32, tag="sd", name="sd")
            nc.scalar.activation(sd[:], vareps[:], AF.Sqrt)
            rstd = fin.tile([S, 1], F32, tag="rstd", name="rstd")
            nc.vector.reciprocal(rstd[:], sd[:])
            t1 = fin.tile([S, HIDDEN], F32, tag="t1", name="t1")
            nc.vector.scalar_tensor_tensor(
                t1[:], xc[:], rstd[:], gamma_rep[:], op0=ALU.mult, op1=ALU.mult
            )
            out_sb = fin.tile([S, HIDDEN], F32, tag="osb", name="out_sb")
            nc.vector.tensor_add(out_sb[:], t1[:], beta_rep[:])
            nc.sync.dma_start(out[:], out_sb[:])


_NC_CACHE = {}


def kernel(hidden_states, mask, Wq, bq, Wv, bv, Wo, bo, gamma, beta):
    if "nc" not in _NC_CACHE:
        _NC_CACHE["nc"] = build_kernel()
    nc = _NC_CACHE["nc"]
    ident = np.eye(128, dtype=np.float32)
    f32 = np.float32

    def cc(a):
        return np.ascontiguousarray(a, dtype=f32)

    in_maps = [
        {
            "hs": cc(hidden_states[b]),
            "mask": cc(mask[b]),
            "wq": cc(Wq),
            "bq": cc(bq),
            "wv": cc(Wv),
            "bv": cc(bv),
            "wo": cc(Wo),
            "bo": cc(bo),
            "gamma": cc(gamma),
            "beta": cc(beta),
            "ident": ident,
        }
        for b in range(N_CORES)
    ]
    res = run_bass_kernel_spmd(nc, in_maps, core_ids=list(range(N_CORES)))
    _NC_CACHE["last_results"] = res
    globals()["_LAST_RESULTS"] = res
    return np.stack([res.results[i]["out"] for i in range(N_CORES)], axis=0)

